# revision 1
# baseline (speedup 1.0000x reference)
"""Trainium2 Bass kernel for nn_DistributionLossWithLabel.

Reference computation (B=8192, C=64):
    lq = log(q); lp = log(p)
    positive[i] = mean_c p[i,c]*(lp[i,c]-lq[i,c])
    a[j]        = sum_c p[j,c]*lp[j,c] / C
    kl[i,j]     = a[j] - (lq @ p^T)[i,j] / C
    negative[i] = sum_j kl[i,j] + sum_j kl[i,j]*(1-L[i,j])
    loss        = sum_i positive[i]/negative[i]

Device reformulation (rows i sharded 8 ways, D = 2 - L shipped from host
transposed as bf16; {1,2} and {0,1} are exact in bf16):
    negative[i] = sum_j kl[i,j]*(2-L[i,j])
                = (D@a)[i] - sum_c (lq[i,c]/C) * (D@p)[i,c]
    [Dp | Da] accumulates on the TensorEngine as paug^T @ D^T where
    paug = [p | a_hi | a_lo] (bf16, with a carried as a hi/lo split to
    kill the bf16 rounding of the dominant term), streamed against D^T
    tiles straight from HBM.  The 8192x8192 KL matrix never exists, the
    VectorEngine only does O(B) epilogue work, and the kernel is bound by
    reading D^T once (16MB/core).
"""

import sys

if "/opt/trn_rl_repo" not in sys.path:
    sys.path.insert(0, "/opt/trn_rl_repo")

import ml_dtypes
import numpy as np

import concourse.bass as bass
import concourse.tile as tile
from concourse import bacc, mybir
from concourse.masks import make_identity

FP = mybir.dt.float32
BF = mybir.dt.bfloat16
F8 = mybir.dt.float8e4
AF = mybir.ActivationFunctionType
ALU = mybir.AluOpType
AX = mybir.AxisListType

B_FULL = 8192
C = 64
N_CORES = 8
NAUG = 66  # 64 p columns + a_hi + a_lo


def build_nc(B=B_FULL, shard=B_FULL // N_CORES, debug=False):
    """Build the single-core SPMD Bass program.

    B: total rows (j extent), multiple of 512.
    shard: rows per core (i extent), multiple of 128.
    """
    assert B % 512 == 0 and shard % 128 == 0
    njc = B // 128           # 128-row j-chunks of p / D^T
    nblk = shard // 128      # 128-row i-blocks
    nhalf = (shard + 511) // 512
    ccpt = 4                 # j-chunks per D^T DMA tile
    assert njc % ccpt == 0
    rcpC = 1.0 / C

    nc = bacc.Bacc("TRN2", target_bir_lowering=False, debug=debug)

    # q/p/p_my arrive pre-chunked from host: [128, nchunks*64] where
    # partition pp, col n*64+c = row n*128+pp, col c — so every input DMA
    # is contiguous per partition (line rate) and rows land on partitions.
    q_d = nc.dram_tensor("q", [128, nblk * 64], FP, kind="ExternalInput")
    p_d = nc.dram_tensor("p", [128, njc * 64], FP, kind="ExternalInput")
    pmy_d = nc.dram_tensor("p_my", [128, nblk * 64], FP, kind="ExternalInput")
    # D^T = (2 - labels)^T for this core's row shard: [B, shard] fp8e4m3
    # ({1,2} are exact in e4m3; the PE takes bf16 weights x fp8 moving)
    lab_d = nc.dram_tensor("labels", [B, shard], F8, kind="ExternalInput")
    out_d = nc.dram_tensor("out", [128, 1], FP, kind="ExternalOutput")

    with tile.TileContext(nc) as tc:
        with (
            tc.tile_pool(name="const", bufs=1) as cp,
            tc.tile_pool(name="lpool", bufs=8) as lp_pool,
            tc.tile_pool(name="spool", bufs=2) as sp,
            tc.tile_pool(name="mps_ps", bufs=1, space="PSUM") as mps_ps,
            tc.tile_pool(name="tr_ps", bufs=2, space="PSUM") as tr_ps,
        ):
            ident = cp.tile([128, 128], FP)
            make_identity(nc, ident[:])

            # ---------------- p prologue -> paug (pipelined quarters) -------
            # Quarter-granular ops + subtile deps let main-loop matmuls on
            # early chunks start while later quarters are still loading.
            P_nat = cp.tile([128, njc * 64], FP)
            LP = cp.tile([128, njc * 64], FP)
            A = cp.tile([128, njc * 64], FP)
            asum = cp.tile([128, njc], FP)  # sum_c p*lp (unscaled)
            ah32 = cp.tile([128, njc], FP)
            alo = cp.tile([128, njc], FP)
            paug = cp.tile([128, njc * NAUG], BF)
            paug_v = paug[:].rearrange("p (n w) -> p n w", w=NAUG)

            # First D^T tile on the fast HWDGE ring before anything else —
            # the first matmuls need it and SWDGE has a slow ramp.
            lab_ap = lab_d.ap()
            Lt0 = lp_pool.tile([128, ccpt, shard], F8, tag="L")
            nc.sync.dma_start(
                out=Lt0[:],
                in_=lab_ap[0 : ccpt * 128, :].rearrange("(cc p) i -> p cc i", p=128),
            )

            NQ = 8
            qw = njc // NQ
            p_ap = p_d.ap()
            for qd in range(NQ):
                ns = slice(qd * qw, (qd + 1) * qw)
                fs = slice(qd * qw * 64, (qd + 1) * qw * 64)
                nc.sync.dma_start(out=P_nat[:, fs], in_=p_ap[:, fs])
                nc.scalar.activation(LP[:, fs], P_nat[:, fs], AF.Ln)
                nc.vector.tensor_tensor(
                    A[:, fs], P_nat[:, fs], LP[:, fs], op=ALU.mult
                )
                nc.vector.reduce_sum(
                    asum[:, ns],
                    A[:, fs].rearrange("p (n c) -> p n c", c=64),
                    axis=AX.X,
                )
                nc.scalar.copy(
                    paug_v[:, ns, 0:64],
                    P_nat[:, fs].rearrange("p (n c) -> p n c", c=64),
                )
                # a_hi = bf16(a), a_lo = bf16(a - a_hi); a = asum/C
                nc.scalar.activation(
                    paug_v[:, ns, 64:65],
                    asum[:, ns].rearrange("p (n o) -> p n o", o=1),
                    AF.Copy,
                    scale=rcpC,
                )
                nc.vector.tensor_copy(
                    ah32[:, ns].rearrange("p (n o) -> p n o", o=1),
                    paug_v[:, ns, 64:65],
                )
                nc.vector.scalar_tensor_tensor(
                    out=alo[:, ns],
                    in0=asum[:, ns],
                    scalar=rcpC,
                    in1=ah32[:, ns],
                    op0=ALU.mult,
                    op1=ALU.subtract,
                )
                nc.scalar.copy(
                    paug_v[:, ns, 65:66],
                    alo[:, ns].rearrange("p (n o) -> p n o", o=1),
                )

            # ---------------- main loop: [Dp|Da]^T += paug^T @ D^T ----------
            mps = mps_ps.tile([128, shard], FP)
            for g in range(njc // ccpt):
                if g == 0:
                    Lt = Lt0
                else:
                    Lt = lp_pool.tile([128, ccpt, shard], F8, tag="L")
                    eng = nc.gpsimd if g % 2 == 0 else nc.sync
                    eng.dma_start(
                        out=Lt[:],
                        in_=lab_ap[
                            g * ccpt * 128 : (g + 1) * ccpt * 128, :
                        ].rearrange("(cc p) i -> p cc i", p=128),
                    )
                for cc in range(ccpt):
                    ch = g * ccpt + cc
                    lw = paug[:, ch * NAUG : (ch + 1) * NAUG]
                    for h in range(nhalf):
                        i0 = h * 512
                        iw = min(512, shard - i0)
                        nc.tensor.matmul(
                            mps[0:NAUG, i0 : i0 + iw],
                            lw,
                            Lt[:, cc, i0 : i0 + iw],
                            start=(ch == 0),
                            stop=(ch == njc - 1),
                        )

            # ---------------- q / positive (overlaps main loop) ------------
            QRAW = cp.tile([128, nblk * 64], FP)
            nc.gpsimd.dma_start(out=QRAW[:], in_=q_d.ap())
            lq = cp.tile([128, nblk * 64], FP)
            nc.scalar.activation(lq[:], QRAW[:], AF.Ln)

            Pmy = cp.tile([128, nblk * 64], FP)
            nc.gpsimd.dma_start(out=Pmy[:], in_=pmy_d.ap())
            LPmy = cp.tile([128, nblk * 64], FP)
            nc.scalar.activation(LPmy[:], Pmy[:], AF.Ln)
            tsub = cp.tile([128, nblk * 64], FP)
            nc.vector.tensor_tensor(tsub[:], LPmy[:], lq[:], op=ALU.subtract)
            pos_sb = cp.tile([128, nblk], FP)
            for blk in range(nblk):
                pscr = sp.tile([128, 64], FP, tag="pscr")
                nc.vector.scalar_tensor_tensor(
                    out=pscr[:],
                    in0=Pmy[:, blk * 64 : (blk + 1) * 64],
                    scalar=rcpC,
                    in1=tsub[:, blk * 64 : (blk + 1) * 64],
                    op0=ALU.mult,
                    op1=ALU.mult,
                    accum_out=pos_sb[:, blk : blk + 1],
                )

            # ---------------- epilogue ----------------
            DpT = cp.tile([128, shard], FP)
            nc.scalar.copy(DpT[0:NAUG, :], mps[0:NAUG, :])
            updp = cp.tile([128, nblk], FP)
            da2 = cp.tile([128, nblk * 2], FP)
            for blk in range(nblk):
                tr = tr_ps.tile([128, NAUG], FP, tag="tr")
                nc.tensor.transpose(
                    tr[:],
                    DpT[0:NAUG, blk * 128 : (blk + 1) * 128],
                    ident[0:NAUG, 0:NAUG],
                )
                escr = sp.tile([128, 64], FP, tag="escr")
                nc.vector.scalar_tensor_tensor(
                    out=escr[:],
                    in0=tr[:, 0:64],
                    scalar=rcpC,
                    in1=lq[:, blk * 64 : (blk + 1) * 64],
                    op0=ALU.mult,
                    op1=ALU.mult,
                    accum_out=updp[:, blk : blk + 1],
                )
                nc.scalar.copy(da2[:, blk * 2 : (blk + 1) * 2], tr[:, 64:66])
            da_sb = cp.tile([128, nblk], FP)
            da2v = da2[:].rearrange("p (n t) -> p n t", t=2)
            nc.vector.tensor_tensor(
                da_sb[:].rearrange("p (n o) -> p n o", o=1),
                da2v[:, :, 0:1],
                da2v[:, :, 1:2],
                op=ALU.add,
            )
            neg8 = cp.tile([128, nblk], FP)
            nc.vector.scalar_tensor_tensor(
                out=neg8[:],
                in0=updp[:],
                scalar=-1.0,
                in1=da_sb[:],
                op0=ALU.mult,
                op1=ALU.add,
            )
            rec8 = cp.tile([128, nblk], FP)
            nc.vector.reciprocal(rec8[:], neg8[:])
            r8 = cp.tile([128, nblk], FP)
            nc.vector.tensor_tensor(r8[:], pos_sb[:], rec8[:], op=ALU.mult)
            out_col = cp.tile([128, 1], FP)
            nc.vector.reduce_sum(out_col[:], r8[:], axis=AX.X)
            nc.sync.dma_start(out=out_d.ap(), in_=out_col[:])

    nc.compile()
    return nc


_NC_CACHE = {}


def _get_nc(B, shard):
    key = (B, shard)
    if key not in _NC_CACHE:
        _NC_CACHE[key] = build_nc(B, shard)
    return _NC_CACHE[key]


def make_dt(labels_shard):
    """(2 - labels)^T as contiguous fp8e4m3 [B, shard]."""
    return (2.0 - labels_shard).T.astype(ml_dtypes.float8_e4m3, order="C")


def chunk_rows(arr):
    """[N, 64] fp32 -> [128, (N/128)*64]: partition pp, col n*64+c = row
    n*128+pp — the on-chip chunked layout, pre-computed on host so the
    DMA is a contiguous line-rate load."""
    n = arr.shape[0] // 128
    return np.ascontiguousarray(
        arr.reshape(n, 128, 64).transpose(1, 0, 2).reshape(128, n * 64)
    )


def make_in_maps(q, p, labels_matrix, n_cores=N_CORES):
    B = q.shape[0]
    shard = B // n_cores
    maps = []
    p_ch = chunk_rows(p)
    for k in range(n_cores):
        s = slice(k * shard, (k + 1) * shard)
        maps.append(
            {
                "q": chunk_rows(q[s]),
                "p": p_ch,
                "p_my": chunk_rows(p[s]),
                "labels": make_dt(labels_matrix[s]),
            }
        )
    return maps


def kernel(q, p, labels_matrix):
    from concourse.bass_utils import run_bass_kernel_spmd

    q = np.asarray(q, dtype=np.float32)
    p = np.asarray(p, dtype=np.float32)
    labels_matrix = np.asarray(labels_matrix, dtype=np.float32)
    B = q.shape[0]
    shard = B // N_CORES
    nc = _get_nc(B, shard)
    in_maps = make_in_maps(q, p, labels_matrix, N_CORES)
    res = run_bass_kernel_spmd(nc, in_maps, core_ids=list(range(N_CORES)))
    total = 0.0
    for r in res.results:
        total += r["out"].astype(np.float64).sum()
    return np.float32(total)

